# revision 5
# baseline (speedup 1.0000x reference)
"""Trainium2 Bass kernel for sparse (sliding-window, GQA, RoPE) attention.

Sharding: 8-way tensor-parallel over heads. Core c owns q-heads 4c..4c+3 and
kv-head c (wq/wk/wv column-parallel, wo row-parallel); each core produces a
full-shape partial output and the host sums the 8 partials (the all-reduce).

v2 vs baseline: all operands bf16 (halves DMA + SBUF; PE rate unchanged at
1 cyc/row), phase-1 projections accumulate 32-deep in PSUM with the
contraction as the outer loop (x streamed through a tiny rotating window, no
SBUF adds), RoPE applied per 512-token chunk straight out of PSUM, big/batched
DMAs (host pre-arranges DRAM layouts so every transfer is one straight 2D
copy), and the output projection runs per-seq right after that seq's
attention. Scores stay f32 in PSUM; exp -> bf16; masks multiply post-exp.
"""
import numpy as np
from contextlib import ExitStack

import ml_dtypes

import concourse.bass as bass
from concourse import bacc
import concourse.mybir as mybir
import concourse.tile as tile
from concourse.bass_utils import run_bass_kernel_spmd

BF = mybir.dt.bfloat16
F32 = mybir.dt.float32
NPBF = ml_dtypes.bfloat16

NCORE = 8
T = 2048              # total tokens (2 seqs x 1024)
DIM = 4096
SEQ = 1024
NSEQ = 2
HD = 128              # head dim
NH = 4                # q heads per core
NKT = DIM // 128      # 32 contraction tiles
QB = 256              # attention q-block width
SCALE = float(HD) ** -0.5
WCOLS = NH * HD + 2 * HD   # 768 projection output cols per core

# per-(seq-local qb) score k-tile lists: (seq-local k-tile index, mask id)
# masks: -1 none, 0: j>=p (C0), 1: j>=p+128 (C1), 2: j<p (F0), 3: j<p+128 (F1)
QB_TILES = [
    [(0, 0), (1, 1)],
    [(0, -1), (1, -1), (2, 0), (3, 1)],
    [(0, 2), (1, 3), (2, -1), (3, -1), (4, 0), (5, 1)],
    [(2, 2), (3, 3), (4, -1), (5, -1), (6, 0), (7, 1)],
]
MAXKT = 6

_NC_CACHE = {}


def _build_nc(reps=1, internal_io=False):
    nc = bacc.Bacc("TRN2", target_bir_lowering=False, debug=False,
                   num_devices=NCORE)
    if internal_io:
        # timing-only variant: big tensors live in device DRAM (no host
        # transfer per run); tiny dummy in/out keep the pjrt contract.
        def dram_in(name, shape, dt):
            return nc.dram_tensor(name, shape, dt).ap()
        dummy_in = nc.declare_dram_parameter("tin", [128, 128], F32, isOutput=False)
        dout = nc.declare_dram_parameter("tout", [128, 128], F32, isOutput=True)
        outp = nc.dram_tensor("outp_i", [T, DIM], BF).ap()
    else:
        def dram_in(name, shape, dt):
            return nc.declare_dram_parameter(name, shape, dt, isOutput=False)
        outp = nc.declare_dram_parameter("outp", [T, DIM], BF, isOutput=True)
    # host pre-arranged layouts (see _host_prep):
    #   xTb[(ch*8+b)*128 + p, kl*512 + c] = x[ch*512 + c, (4b+kl)*128 + p]
    #   wqkvTr[p, k*768 + j] = wqkv[j, k*128 + p]
    #   woTr[p, h*4096 + o]  = wo[o, core*512 + h*128 + p]
    xTb = dram_in("xTb", [NKT * 128, 4 * 512], BF)
    wqkvTr = dram_in("wqkvTr", [128, NKT * WCOLS], BF)
    woTr = dram_in("woTr", [128, NH * DIM], BF)
    cosT = dram_in("cosT", [128, T], BF)
    sinT = dram_in("sinT", [128, T], BF)
    aux = dram_in("aux", [128, 8 * QB + 129], BF)

    with tile.TileContext(nc) as tc, ExitStack() as top:
        persist = top.enter_context(tc.tile_pool(name="persist", bufs=1))
        if internal_io:
            dtile = persist.tile([128, 128], F32, name="dtile", tag="dtile")
            nc.sync.dma_start(dtile[:], dummy_in[:])
            nc.sync.dma_start(dout[:], dtile[:])

        aux_sb = persist.tile([128, 8 * QB + 129], BF, name="aux_sb", tag="aux")
        nc.sync.dma_start(aux_sb[:], aux[:])
        mask_sb = aux_sb[:, 0:8 * QB]
        ident = aux_sb[:, 8 * QB:8 * QB + 128]
        ones = aux_sb[:, 8 * QB + 128:8 * QB + 129]

        for _rep in range(reps):
         with ExitStack() as rep:
            pq = rep.enter_context(tc.tile_pool(name="pq", bufs=1))
            p1 = rep.enter_context(tc.tile_pool(name="p1", bufs=1))
            p2 = rep.enter_context(tc.tile_pool(name="p2", bufs=1))

            # cross-phase bf16 tiles
            QTps = [[pq.tile([128, 2 * SEQ], BF, name=f"QT{p_}_{s}",
                             tag=f"QT{p_}_{s}") for s in range(2)]
                    for p_ in range(2)]
            KTs = [pq.tile([128, SEQ], BF, name=f"KT{s}", tag=f"KT{s}")
                   for s in range(2)]
            Vc = [pq.tile([128, 512], BF, name=f"Vc{ch}", tag=f"Vc{ch}")
                  for ch in range(4)]
            attnTs = [[pq.tile([128, 2 * SEQ], BF, name=f"AT{p_}_{s}",
                               tag=f"AT{p_}_{s}") for s in range(2)]
                      for p_ in range(2)]

            # ---------------- phase 1: projections + rope -----------------
            w_sb = p1.tile([128, NKT * WCOLS], BF, name="w_sb", tag="w_sb")
            for wp in range(8):
                nc.sync.dma_start(
                    w_sb[:, wp * 4 * WCOLS:(wp + 1) * 4 * WCOLS],
                    wqkvTr[:, wp * 4 * WCOLS:(wp + 1) * 4 * WCOLS])
            cos_sb = p1.tile([128, T], BF, name="cos_sb", tag="cos")
            sin_sb = p1.tile([128, T], BF, name="sin_sb", tag="sin")
            nc.sync.dma_start(cos_sb[:], cosT[:])
            nc.sync.dma_start(sin_sb[:], sinT[:])

            with tc.tile_pool(name="ps1", bufs=1, space="PSUM") as ps1:
                for ch in range(4):
                    s, loc = divmod(ch, 2)
                    csl = slice(ch * 512, (ch + 1) * 512)
                    xs = []
                    for b in range(8):
                        t_ = p1.tile([128, 4 * 512], BF, name=f"xs{ch}_{b}",
                                     tag="xs", bufs=2)
                        nc.sync.dma_start(
                            t_[:], xTb[(ch * 8 + b) * 128:(ch * 8 + b + 1) * 128, :])
                        xs.append(t_)
                    accs = [ps1.tile([128, 512], F32, name=f"acc{ch}_{g}",
                                     tag="acc", bufs=7) for g in range(6)]
                    for k in range(NKT):
                        b, kl = divmod(k, 4)
                        mv = xs[b][:, kl * 512:(kl + 1) * 512]
                        for g in range(6):
                            nc.tensor.matmul(
                                accs[g][:], w_sb[:, k * WCOLS + g * 128:
                                                 k * WCOLS + (g + 1) * 128],
                                mv, start=(k == 0), stop=(k == NKT - 1))
                    # RoPE on Q heads (g 0..3) and K (g 4), from PSUM.
                    # sin_sb holds [+sin; -sin] so rotated =
                    # acc*cos + swap_halves(acc*sin_sgn).
                    b_all = p1.tile([128, 5 * 512], BF, name="b_all",
                                    tag="b_all", bufs=1)
                    dests = []
                    for g in range(5):
                        if g < 4:
                            p_, e = divmod(g, 2)
                            dest = QTps[p_][s][:, loc * 1024 + e:
                                               (loc + 1) * 1024:2]
                        else:
                            dest = KTs[s][:, loc * 512:(loc + 1) * 512]
                        dests.append(dest)
                        nc.vector.tensor_mul(b_all[:, g * 512:(g + 1) * 512],
                                             accs[g][:], sin_sb[:, csl])
                        nc.vector.tensor_mul(dest, accs[g][:], cos_sb[:, csl])
                    bs_all = p1.tile([128, 5 * 512], BF, name="bs_all",
                                     tag="bs_all", bufs=1)
                    nc.scalar.dma_start(bs_all[0:64, :], b_all[64:128, :])
                    nc.scalar.dma_start(bs_all[64:128, :], b_all[0:64, :])
                    for g in range(5):
                        nc.vector.tensor_add(dests[g], dests[g],
                                             bs_all[:, g * 512:(g + 1) * 512])
                    # V^T -> V natural via PE transpose
                    vtc = p1.tile([128, 512], BF, name="vtc", tag="vtc", bufs=1)
                    nc.scalar.copy(vtc[:], accs[5][:])
                    tr = ps1.tile([128, 512], BF, name="tr", tag="tr", bufs=1)
                    for q4 in range(4):
                        nc.tensor.transpose(tr[:, q4 * 128:(q4 + 1) * 128],
                                            vtc[:, q4 * 128:(q4 + 1) * 128],
                                            ident)
                    nc.vector.tensor_copy(Vc[ch][:], tr[:])

            # ---------------- phase 2+3: attention + output proj ----------
            wo_sb = p2.tile([128, NH * DIM], BF, name="wo_sb", tag="wo_sb")
            for h in range(NH):
                nc.sync.dma_start(wo_sb[:, h * DIM:(h + 1) * DIM],
                                  woTr[:, h * DIM:(h + 1) * DIM])
            gp_ctr = 0
            with tc.tile_pool(name="psA", bufs=1, space="PSUM") as psA:
                for s in range(NSEQ):
                    lrows = [p2.tile([1, 2 * SEQ], BF, name=f"lrow{p_}",
                                     tag=f"lrow{p_}", bufs=1)
                             for p_ in range(2)]
                    for qb in range(SEQ // QB):
                        tiles = QB_TILES[qb]
                        n = len(tiles)
                        qsl = slice(2 * qb * QB, 2 * (qb + 1) * QB)
                        for p_ in range(2):
                            pt = p2.tile([128, MAXKT * 2 * QB], BF, name="pt",
                                         tag="pt", bufs=2)
                            for gi in range(0, n, 2):
                                grp = tiles[gi:gi + 2]
                                sc = psA.tile([128, 1024], F32, name="sc",
                                              tag="sc", bufs=2)
                                for i, (j, _) in enumerate(grp):
                                    nc.tensor.matmul(
                                        sc[:, i * 512:(i + 1) * 512],
                                        KTs[s][:, j * 128:(j + 1) * 128],
                                        QTps[p_][s][:, qsl],
                                        start=True, stop=True)
                                w = len(grp) * 512
                                nc.scalar.activation(
                                    pt[:, gi * 512:gi * 512 + w], sc[:, 0:w],
                                    mybir.ActivationFunctionType.Exp,
                                    scale=SCALE)
                            for i, (j, mi) in enumerate(tiles):
                                if mi < 0:
                                    continue
                                psl_ = pt[:, i * 512:(i + 1) * 512]
                                msl_ = mask_sb[:, mi * 512:(mi + 1) * 512]
                                eng = nc.gpsimd if (gp_ctr % 3 == 2) else nc.vector
                                gp_ctr += 1
                                eng.tensor_mul(psl_, psl_, msl_)
                            ovlv = psA.tile([128, 1024], F32, name="ovlv",
                                            tag="ovlv", bufs=1)
                            ov = ovlv[:, 0:512]
                            lvv = ovlv[0:1, 512:1024]
                            for i, (j, _) in enumerate(tiles):
                                nc.tensor.matmul(
                                    ov, Vc[s * 2 + j // 4][:, (j % 4) * 128:
                                                           (j % 4 + 1) * 128],
                                    pt[:, i * 512:(i + 1) * 512],
                                    start=(i == 0), stop=(i == n - 1))
                            for i, (j, _) in enumerate(tiles):
                                nc.tensor.matmul(
                                    lvv, ones, pt[:, i * 512:(i + 1) * 512],
                                    start=(i == 0), stop=(i == n - 1))
                            nc.vector.tensor_copy(attnTs[p_][s][:, qsl], ov)
                            nc.scalar.copy(
                                lrows[p_][0:1, qb * 512:(qb + 1) * 512], lvv)
                    for p_ in range(2):
                        with nc.allow_low_precision(reason="softmax denom scale"):
                            nc.vector.reciprocal(lrows[p_][:], lrows[p_][:])
                        lb = p2.tile([128, 2 * SEQ], BF, name="lb", tag="lb",
                                     bufs=2)
                        nc.gpsimd.partition_broadcast(lb[:], lrows[p_][:])
                        nc.vector.tensor_mul(attnTs[p_][s][:],
                                             attnTs[p_][s][:], lb[:])
                    # ------- output projection for seq s -------
                    for tl in range(8):
                        tb = s * 8 + tl
                        for sh in range(2):
                            stg = p2.tile([128, 2048], BF, name="stg",
                                          tag="stg", bufs=2)
                            for cc in range(4):
                                chn = sh * 4 + cc
                                oc = psA.tile([128, 512], F32, name="oc",
                                              tag="oc", bufs=2)
                                for h in range(NH):
                                    st = attnTs[h // 2][s][:,
                                        tl * 256 + (h % 2):(tl + 1) * 256:2]
                                    nc.tensor.matmul(
                                        oc[:],
                                        st,
                                        wo_sb[:, h * DIM + chn * 512:
                                              h * DIM + (chn + 1) * 512],
                                        start=(h == 0), stop=(h == NH - 1))
                                dsl = stg[:, cc * 512:(cc + 1) * 512]
                                if cc % 2 == 0:
                                    nc.scalar.copy(dsl, oc[:])
                                else:
                                    nc.vector.tensor_copy(dsl, oc[:])
                            nc.sync.dma_start(
                                outp[tb * 128:(tb + 1) * 128,
                                     sh * 2048:(sh + 1) * 2048], stg[:])

    nc.compile()
    return nc


def _get_nc():
    if "nc" not in _NC_CACHE:
        _NC_CACHE["nc"] = _build_nc()
    return _NC_CACHE["nc"]


def _host_prep(x, cos, sin, wq, wk, wv, wo):
    perm = np.concatenate([np.arange(0, 128, 2), np.arange(1, 128, 2)])
    wq_p = wq.reshape(32, 128, DIM)[:, perm, :].reshape(32 * 128, DIM)
    wk_p = wk.reshape(8, 128, DIM)[:, perm, :].reshape(8 * 128, DIM)
    xT = np.ascontiguousarray(x.T)  # [DIM, T]
    # xTb[(ch*8+b)*128 + p, kl*512 + c] = xT[(4b+kl)*128 + p, ch*512 + c]
    xTb = np.ascontiguousarray(
        xT.reshape(8, 4, 128, 4, 512).transpose(3, 0, 2, 1, 4)
        .reshape(NKT * 128, 4 * 512)).astype(NPBF)
    cosT = np.vstack([cos.T, cos.T]).astype(NPBF)
    sinT = np.vstack([sin.T, -sin.T]).astype(NPBF)
    p = np.arange(128)[:, None]
    j = np.arange(QB)[None, :]
    masks = [(j >= p), (j >= p + 128), (j < p), (j < p + 128)]
    aux = np.concatenate(
        [np.repeat(m, 2, axis=1) for m in masks]
        + [np.eye(128, dtype=bool), np.ones((128, 1), dtype=bool)],
        axis=1).astype(NPBF)
    in_maps = []
    for c in range(NCORE):
        wqkv = np.concatenate([
            wq_p[c * 512:(c + 1) * 512],
            wk_p[c * 128:(c + 1) * 128],
            wv[c * 128:(c + 1) * 128]], axis=0)  # [768, DIM]
        wqkvTr = np.ascontiguousarray(
            wqkv.T.reshape(NKT, 128, WCOLS).transpose(1, 0, 2)
            .reshape(128, NKT * WCOLS)).astype(NPBF)
        wos = wo[:, c * 512:(c + 1) * 512].T  # [512 feat, DIM out]
        woTr = np.ascontiguousarray(
            wos.reshape(NH, 128, DIM).transpose(1, 0, 2)
            .reshape(128, NH * DIM)).astype(NPBF)
        in_maps.append({
            "xTb": xTb, "wqkvTr": wqkvTr, "woTr": woTr,
            "cosT": cosT, "sinT": sinT, "aux": aux,
        })
    return in_maps


def kernel(x, cos, sin, wq, wk, wv, wo, n_seqs):
    x = np.asarray(x, dtype=np.float32)
    cos = np.asarray(cos, dtype=np.float32)
    sin = np.asarray(sin, dtype=np.float32)
    wq = np.asarray(wq, dtype=np.float32)
    wk = np.asarray(wk, dtype=np.float32)
    wv = np.asarray(wv, dtype=np.float32)
    wo = np.asarray(wo, dtype=np.float32)
    assert int(n_seqs) == NSEQ and x.shape == (T, DIM)

    nc = _get_nc()
    in_maps = _host_prep(x, cos, sin, wq, wk, wv, wo)
    res = run_bass_kernel_spmd(nc, in_maps, list(range(NCORE))).results
    out = np.zeros((T, DIM), dtype=np.float32)
    for c in range(NCORE):
        out += res[c]["outp"].astype(np.float32)
    return out


# revision 8
# speedup vs baseline: 1.3664x; 1.3664x over previous
"""Trainium2 Bass kernel for sparse (sliding-window, GQA, RoPE) attention.

Sharding: 8-way tensor-parallel over heads. Core c owns q-heads 4c..4c+3 and
kv-head c (wq/wk/wv column-parallel, wo row-parallel); each core produces a
full-shape partial output and the host sums the 8 partials (the all-reduce).

v2 vs baseline: all operands bf16 (halves DMA + SBUF; PE rate unchanged at
1 cyc/row), phase-1 projections accumulate 32-deep in PSUM with the
contraction as the outer loop (x streamed through a tiny rotating window, no
SBUF adds), RoPE applied per 512-token chunk straight out of PSUM, big/batched
DMAs (host pre-arranges DRAM layouts so every transfer is one straight 2D
copy), and the output projection runs per-seq right after that seq's
attention. Scores stay f32 in PSUM; exp -> bf16; masks multiply post-exp.
"""
import numpy as np
from contextlib import ExitStack

import ml_dtypes

import concourse.bass as bass
from concourse import bacc
import concourse.mybir as mybir
import concourse.tile as tile
from concourse.bass_utils import run_bass_kernel_spmd

BF = mybir.dt.bfloat16
F32 = mybir.dt.float32
NPBF = ml_dtypes.bfloat16

NCORE = 8
T = 2048              # total tokens (2 seqs x 1024)
DIM = 4096
SEQ = 1024
NSEQ = 2
HD = 128              # head dim
NH = 4                # q heads per core
NKT = DIM // 128      # 32 contraction tiles
QB = 256              # attention q-block width
SCALE = float(HD) ** -0.5
WCOLS = NH * HD + 2 * HD   # 768 projection output cols per core

# per-(seq-local qb) score k-tile lists: (seq-local k-tile index, mask id)
# masks: -1 none, 0: j>=p (C0), 1: j>=p+128 (C1), 2: j<p (F0), 3: j<p+128 (F1)
QB_TILES = [
    [(0, 0), (1, 1)],
    [(0, -1), (1, -1), (2, 0), (3, 1)],
    [(0, 2), (1, 3), (2, -1), (3, -1), (4, 0), (5, 1)],
    [(2, 2), (3, 3), (4, -1), (5, -1), (6, 0), (7, 1)],
]
MAXKT = 6

_NC_CACHE = {}


def _build_nc(reps=1, internal_io=False):
    nc = bacc.Bacc("TRN2", target_bir_lowering=False, debug=False,
                   num_devices=NCORE)
    if internal_io:
        # timing-only variant: big tensors live in device DRAM (no host
        # transfer per run); tiny dummy in/out keep the pjrt contract.
        def dram_in(name, shape, dt):
            return nc.dram_tensor(name, shape, dt).ap()
        dummy_in = nc.declare_dram_parameter("tin", [128, 128], F32, isOutput=False)
        dout = nc.declare_dram_parameter("tout", [128, 128], F32, isOutput=True)
        outp = nc.dram_tensor("outp_i", [T, DIM], BF).ap()
    else:
        def dram_in(name, shape, dt):
            return nc.declare_dram_parameter(name, shape, dt, isOutput=False)
        outp = nc.declare_dram_parameter("outp", [T, DIM], BF, isOutput=True)
    # host pre-arranged layouts (see _host_prep):
    #   xTb[(ch*8+b)*128 + p, kl*512 + c] = x[ch*512 + c, (4b+kl)*128 + p]
    #   wqkvTr[p, k*768 + j] = wqkv[j, k*128 + p]
    #   woTr[p, h*4096 + o]  = wo[o, core*512 + h*128 + p]
    xTb = dram_in("xTb", [NKT * 128, 4 * 512], BF)
    wqkvTr = dram_in("wqkvTr", [128, NKT * WCOLS], BF)
    woTr = dram_in("woTr", [128, NH * DIM], BF)
    cosT = dram_in("cosT", [128, T], BF)
    sinT = dram_in("sinT", [128, T], BF)
    aux = dram_in("aux", [128, 8 * QB + 129], BF)

    with tile.TileContext(nc) as tc, ExitStack() as top:
        persist = top.enter_context(tc.tile_pool(name="persist", bufs=1))
        if internal_io:
            dtile = persist.tile([128, 128], F32, name="dtile", tag="dtile")
            nc.sync.dma_start(dtile[:], dummy_in[:])
            nc.sync.dma_start(dout[:], dtile[:])

        aux_sb = persist.tile([128, 8 * QB + 129], BF, name="aux_sb", tag="aux")
        nc.scalar.dma_start(aux_sb[:], aux[:])
        mask_sb = aux_sb[:, 0:8 * QB]
        ident = aux_sb[:, 8 * QB:8 * QB + 128]
        ones = aux_sb[:, 8 * QB + 128:8 * QB + 129]

        for _rep in range(reps):
         with ExitStack() as rep:
            pq = rep.enter_context(tc.tile_pool(name="pq", bufs=1))
            p1 = rep.enter_context(tc.tile_pool(name="p1", bufs=1))
            p2 = rep.enter_context(tc.tile_pool(name="p2", bufs=1))

            # cross-phase bf16 tiles
            QTps = [[pq.tile([128, 2 * SEQ], BF, name=f"QT{p_}_{s}",
                             tag=f"QT{p_}_{s}") for s in range(2)]
                    for p_ in range(2)]
            KTs = [pq.tile([128, SEQ], BF, name=f"KT{s}", tag=f"KT{s}")
                   for s in range(2)]
            Vc = [pq.tile([128, 512], BF, name=f"Vc{ch}", tag=f"Vc{ch}")
                  for ch in range(4)]
            attnTs = [[pq.tile([128, 2 * SEQ], BF, name=f"AT{p_}_{s}",
                               tag=f"AT{p_}_{s}") for s in range(2)]
                      for p_ in range(2)]

            # ---------------- phase 1: projections + rope -----------------
            # cos/sin/aux go via the Act HWDGE queue so the SP queue starts
            # with the first w/x tiles the PE needs (small start bubble).
            w_sb = p1.tile([128, NKT * WCOLS], BF, name="w_sb", tag="w_sb")
            cos_sb = p1.tile([128, T], BF, name="cos_sb", tag="cos")
            sin_sb = p1.tile([128, T], BF, name="sin_sb", tag="sin")
            nc.scalar.dma_start(cos_sb[:], cosT[:])
            nc.scalar.dma_start(sin_sb[:], sinT[:])

            def ld_w(k0, k1):
                nc.sync.dma_start(w_sb[:, k0 * WCOLS:k1 * WCOLS],
                                  wqkvTr[:, k0 * WCOLS:k1 * WCOLS])

            with tc.tile_pool(name="ps1", bufs=1, space="PSUM") as ps1:
                for ch in range(4):
                    s, loc = divmod(ch, 2)
                    csl = slice(ch * 512, (ch + 1) * 512)
                    xs = []
                    for b in range(8):
                        if ch == 0:
                            # interleave w parts with the first chunk's x
                            # blocks (matching k-ranges) so matmul k can
                            # start as soon as its operands land.
                            if b == 0:
                                ld_w(0, 1)
                            elif b == 1:
                                ld_w(1, 4)
                            else:
                                ld_w((b - 1) * 4, b * 4)
                        t_ = p1.tile([128, 4 * 512], BF, name=f"xs{ch}_{b}",
                                     tag="xs", bufs=2)
                        nc.sync.dma_start(
                            t_[:], xTb[(ch * 8 + b) * 128:(ch * 8 + b + 1) * 128, :])
                        xs.append(t_)
                    if ch == 0:
                        ld_w(28, 32)
                    accs = [ps1.tile([128, 512], F32, name=f"acc{ch}_{g}",
                                     tag="acc", bufs=7) for g in range(6)]
                    for k in range(NKT):
                        b, kl = divmod(k, 4)
                        mv = xs[b][:, kl * 512:(kl + 1) * 512]
                        for g in range(6):
                            nc.tensor.matmul(
                                accs[g][:], w_sb[:, k * WCOLS + g * 128:
                                                 k * WCOLS + (g + 1) * 128],
                                mv, start=(k == 0), stop=(k == NKT - 1))
                    # RoPE on Q heads (g 0..3) and K (g 4), from PSUM.
                    # sin_sb holds [+sin; -sin] so rotated =
                    # acc*cos + swap_halves(acc*sin_sgn).
                    b_all = p1.tile([128, 5 * 512], BF, name="b_all",
                                    tag="b_all", bufs=1)
                    dests = []
                    for g in range(5):
                        if g < 4:
                            p_, e = divmod(g, 2)
                            dest = QTps[p_][s][:, loc * 1024 + e:
                                               (loc + 1) * 1024:2]
                        else:
                            dest = KTs[s][:, loc * 512:(loc + 1) * 512]
                        dests.append(dest)
                        nc.vector.tensor_mul(b_all[:, g * 512:(g + 1) * 512],
                                             accs[g][:], sin_sb[:, csl])
                        nc.vector.tensor_mul(dest, accs[g][:], cos_sb[:, csl])
                    bs_all = p1.tile([128, 5 * 512], BF, name="bs_all",
                                     tag="bs_all", bufs=1)
                    nc.scalar.dma_start(bs_all[0:64, :], b_all[64:128, :])
                    nc.scalar.dma_start(bs_all[64:128, :], b_all[0:64, :])
                    for g in range(5):
                        nc.vector.tensor_add(dests[g], dests[g],
                                             bs_all[:, g * 512:(g + 1) * 512])
                    # V^T -> V natural via PE transpose
                    vtc = p1.tile([128, 512], BF, name="vtc", tag="vtc", bufs=1)
                    nc.scalar.copy(vtc[:], accs[5][:])
                    tr = ps1.tile([128, 512], BF, name="tr", tag="tr", bufs=1)
                    for q4 in range(4):
                        nc.tensor.transpose(tr[:, q4 * 128:(q4 + 1) * 128],
                                            vtc[:, q4 * 128:(q4 + 1) * 128],
                                            ident)
                    nc.vector.tensor_copy(Vc[ch][:], tr[:])

            # ---------------- phase 2+3: attention + output proj ----------
            wo_sb = p2.tile([128, NH * DIM], BF, name="wo_sb", tag="wo_sb")
            for h in range(NH):
                nc.sync.dma_start(wo_sb[:, h * DIM:(h + 1) * DIM],
                                  woTr[:, h * DIM:(h + 1) * DIM])
            gp_ctr = 0
            with tc.tile_pool(name="psA", bufs=1, space="PSUM") as psA:
                for s in range(NSEQ):
                    lrows = [p2.tile([1, 2 * SEQ], BF, name=f"lrow{p_}",
                                     tag=f"lrow{p_}", bufs=1)
                             for p_ in range(2)]
                    for qb in range(SEQ // QB):
                        tiles = QB_TILES[qb]
                        n = len(tiles)
                        qsl = slice(2 * qb * QB, 2 * (qb + 1) * QB)
                        for p_ in range(2):
                            pt = p2.tile([128, MAXKT * 2 * QB], BF, name="pt",
                                         tag="pt", bufs=2)
                            for gi in range(0, n, 2):
                                grp = tiles[gi:gi + 2]
                                sc = psA.tile([128, 1024], F32, name="sc",
                                              tag="sc", bufs=2)
                                for i, (j, _) in enumerate(grp):
                                    nc.tensor.matmul(
                                        sc[:, i * 512:(i + 1) * 512],
                                        KTs[s][:, j * 128:(j + 1) * 128],
                                        QTps[p_][s][:, qsl],
                                        start=True, stop=True)
                                w = len(grp) * 512
                                nc.scalar.activation(
                                    pt[:, gi * 512:gi * 512 + w], sc[:, 0:w],
                                    mybir.ActivationFunctionType.Exp,
                                    scale=SCALE)
                            for i, (j, mi) in enumerate(tiles):
                                if mi < 0:
                                    continue
                                psl_ = pt[:, i * 512:(i + 1) * 512]
                                msl_ = mask_sb[:, mi * 512:(mi + 1) * 512]
                                eng = nc.gpsimd if (gp_ctr % 3 == 2) else nc.vector
                                gp_ctr += 1
                                eng.tensor_mul(psl_, psl_, msl_)
                            ovlv = psA.tile([128, 1024], F32, name="ovlv",
                                            tag="ovlv", bufs=1)
                            ov = ovlv[:, 0:512]
                            lvv = ovlv[0:1, 512:1024]
                            for i, (j, _) in enumerate(tiles):
                                nc.tensor.matmul(
                                    ov, Vc[s * 2 + j // 4][:, (j % 4) * 128:
                                                           (j % 4 + 1) * 128],
                                    pt[:, i * 512:(i + 1) * 512],
                                    start=(i == 0), stop=(i == n - 1))
                            for i, (j, _) in enumerate(tiles):
                                nc.tensor.matmul(
                                    lvv, ones, pt[:, i * 512:(i + 1) * 512],
                                    start=(i == 0), stop=(i == n - 1))
                            nc.vector.tensor_copy(attnTs[p_][s][:, qsl], ov)
                            nc.scalar.copy(
                                lrows[p_][0:1, qb * 512:(qb + 1) * 512], lvv)
                    for p_ in range(2):
                        with nc.allow_low_precision(reason="softmax denom scale"):
                            nc.vector.reciprocal(lrows[p_][:], lrows[p_][:])
                        lb = p2.tile([128, 2 * SEQ], BF, name="lb", tag="lb",
                                     bufs=2)
                        nc.gpsimd.partition_broadcast(lb[:], lrows[p_][:])
                        nc.vector.tensor_mul(attnTs[p_][s][:],
                                             attnTs[p_][s][:], lb[:])
                # ------- output projection (all seqs, PE-dense) -------
                for s in range(NSEQ):
                    for tl in range(8):
                        tb = s * 8 + tl
                        for sh in range(2):
                            stg = p2.tile([128, 2048], BF, name="stg",
                                          tag="stg", bufs=2)
                            for cc in range(4):
                                chn = sh * 4 + cc
                                oc = psA.tile([128, 512], F32, name="oc",
                                              tag="oc", bufs=2)
                                for h in range(NH):
                                    st = attnTs[h // 2][s][:,
                                        tl * 256 + (h % 2):(tl + 1) * 256:2]
                                    nc.tensor.matmul(
                                        oc[:],
                                        st,
                                        wo_sb[:, h * DIM + chn * 512:
                                              h * DIM + (chn + 1) * 512],
                                        start=(h == 0), stop=(h == NH - 1))
                                dsl = stg[:, cc * 512:(cc + 1) * 512]
                                if cc % 2 == 0:
                                    nc.scalar.copy(dsl, oc[:])
                                else:
                                    nc.vector.tensor_copy(dsl, oc[:])
                            nc.sync.dma_start(
                                outp[tb * 128:(tb + 1) * 128,
                                     sh * 2048:(sh + 1) * 2048], stg[:])

    nc.compile()
    return nc


def _get_nc():
    if "nc" not in _NC_CACHE:
        _NC_CACHE["nc"] = _build_nc()
    return _NC_CACHE["nc"]


def _host_prep(x, cos, sin, wq, wk, wv, wo):
    perm = np.concatenate([np.arange(0, 128, 2), np.arange(1, 128, 2)])
    wq_p = wq.reshape(32, 128, DIM)[:, perm, :].reshape(32 * 128, DIM)
    wk_p = wk.reshape(8, 128, DIM)[:, perm, :].reshape(8 * 128, DIM)
    xT = np.ascontiguousarray(x.T)  # [DIM, T]
    # xTb[(ch*8+b)*128 + p, kl*512 + c] = xT[(4b+kl)*128 + p, ch*512 + c]
    xTb = np.ascontiguousarray(
        xT.reshape(8, 4, 128, 4, 512).transpose(3, 0, 2, 1, 4)
        .reshape(NKT * 128, 4 * 512)).astype(NPBF)
    cosT = np.vstack([cos.T, cos.T]).astype(NPBF)
    sinT = np.vstack([sin.T, -sin.T]).astype(NPBF)
    p = np.arange(128)[:, None]
    j = np.arange(QB)[None, :]
    masks = [(j >= p), (j >= p + 128), (j < p), (j < p + 128)]
    aux = np.concatenate(
        [np.repeat(m, 2, axis=1) for m in masks]
        + [np.eye(128, dtype=bool), np.ones((128, 1), dtype=bool)],
        axis=1).astype(NPBF)
    in_maps = []
    for c in range(NCORE):
        wqkv = np.concatenate([
            wq_p[c * 512:(c + 1) * 512],
            wk_p[c * 128:(c + 1) * 128],
            wv[c * 128:(c + 1) * 128]], axis=0)  # [768, DIM]
        wqkvTr = np.ascontiguousarray(
            wqkv.T.reshape(NKT, 128, WCOLS).transpose(1, 0, 2)
            .reshape(128, NKT * WCOLS)).astype(NPBF)
        wos = wo[:, c * 512:(c + 1) * 512].T  # [512 feat, DIM out]
        woTr = np.ascontiguousarray(
            wos.reshape(NH, 128, DIM).transpose(1, 0, 2)
            .reshape(128, NH * DIM)).astype(NPBF)
        in_maps.append({
            "xTb": xTb, "wqkvTr": wqkvTr, "woTr": woTr,
            "cosT": cosT, "sinT": sinT, "aux": aux,
        })
    return in_maps


def kernel(x, cos, sin, wq, wk, wv, wo, n_seqs):
    x = np.asarray(x, dtype=np.float32)
    cos = np.asarray(cos, dtype=np.float32)
    sin = np.asarray(sin, dtype=np.float32)
    wq = np.asarray(wq, dtype=np.float32)
    wk = np.asarray(wk, dtype=np.float32)
    wv = np.asarray(wv, dtype=np.float32)
    wo = np.asarray(wo, dtype=np.float32)
    assert int(n_seqs) == NSEQ and x.shape == (T, DIM)

    nc = _get_nc()
    in_maps = _host_prep(x, cos, sin, wq, wk, wv, wo)
    res = run_bass_kernel_spmd(nc, in_maps, list(range(NCORE))).results
    out = np.zeros((T, DIM), dtype=np.float32)
    for c in range(NCORE):
        out += res[c]["outp"].astype(np.float32)
    return out
